# revision 1
# baseline (speedup 1.0000x reference)
"""Trainium2 Bass kernel for nn_PlasticityModelMoE (8-core SPMD).

Strategy:
  Phase 1 (units tensor-parallel, 256 units/core): w_mod = w*sigmoid(delay),
    branch+gate logits via one fused matmul per 128-row batch tile (biases added
    through a ones-row K step), gate softmax, z = sum_b gate_b*branch_b,
    a = relu(z*conn*mask), 8-way activation blend (a>=0 collapses elu/relu/selu
    to linear terms; Silu/Mish/Gelu via ACT LUTs).
  AllGather of the PE-transposed blendT (bf16, 1MB/rank).
  Phase 3/4 (memory-rows tensor-parallel, 1024 rows/core), fused per 512-column
    batch chunk: logitsT = read_W_shard x blendT, E = exp(logitsT + read_b),
    [read_partial | s] = E @ [memory_shard | 1], then one f32 ReduceScatter over
    batch rows and a divide; each core emits its 256-row output shard.
"""
import numpy as np
from contextlib import ExitStack

import concourse.bass as bass
import concourse.mybir as mybir
import concourse.tile as tile
from concourse import bacc
from concourse.bass_utils import run_bass_kernel_spmd
from concourse.masks import make_identity

F32 = mybir.dt.float32
BF16 = mybir.dt.bfloat16
AF = mybir.ActivationFunctionType
ALU = mybir.AluOpType
AX = mybir.AxisListType

KC = 8
N, D, U, NB, M, MD = 2048, 1024, 2048, 4, 8192, 1024
US = U // KC          # 256 units per core
MS = M // KC          # 1024 memory rows per core
NS = N // KC          # 256 output rows per core
NT = N // 128         # 16 batch tiles
DK = D // 128         # 8 k-tiles over D
UK = U // 128         # 16 k-tiles over U
MK = MS // 128        # 8 k-tiles over memory shard
UBF = US * NB         # 1024 branch columns per core
SELU_SCALE = 1.0507009873554805

_CMAT = np.array([
    [5.0000238e-01, 2.4987496e-01, 1.0582031e-03, -2.4046743e-02, 4.1678566e-03],
    [0.0, 1.0, 0.0, 0.0, 0.0],
    [-7.2632770e-06, 9.9976927e-01, 9.2018498e-03, -3.9401752e-01, 1.4669961e-01],
    [0.0, 1.0, 0.0, 0.0, 0.0],
    [8.6798245e-06, 4.9957812e-01, 2.5321743e-01, -8.1970906e-03, -1.3558048e-02],
    [3.9388153e-05, 4.9807969e-01, 4.1364601e-01, -3.7666172e-02, -3.2796454e-02],
    [0.0, 1.0507009873554805, 0.0, 0.0, 0.0],
    [3.1482985e-05, 5.9846270e-01, 3.3178753e-01, -4.6201140e-02, -1.9015398e-02],
    [0.0, 0.0, 0.0, 0.0, 0.0],
], dtype=np.float32)

_cache = {}


def _build():
    nc = bacc.Bacc(num_devices=KC)

    x_d = nc.dram_tensor("x", [N, D], F32, kind="ExternalInput")
    wd_d = nc.dram_tensor("wd", [D, UBF + NB], F32, kind="ExternalInput")
    dd_d = nc.dram_tensor("dd", [D, UBF], F32, kind="ExternalInput")
    bias_d = nc.dram_tensor("bias", [UBF + NB], F32, kind="ExternalInput")
    na_d = nc.dram_tensor("na", [U], F32, kind="ExternalInput")
    cw1_d = nc.dram_tensor("cw1", [U, 32], F32, kind="ExternalInput")
    cb1_d = nc.dram_tensor("cb1", [32], F32, kind="ExternalInput")
    cw2_d = nc.dram_tensor("cw2", [32, US], F32, kind="ExternalInput")
    cb2_d = nc.dram_tensor("cb2", [US], F32, kind="ExternalInput")
    mask_d = nc.dram_tensor("maskv", [US], F32, kind="ExternalInput")
    actw_d = nc.dram_tensor("actw", [9], F32, kind="ExternalInput")
    rw_d = nc.dram_tensor("rw", [U, MS], F32, kind="ExternalInput")
    rb_d = nc.dram_tensor("rb", [MS], F32, kind="ExternalInput")
    mem_d = nc.dram_tensor("mem", [MS, MD], F32, kind="ExternalInput")
    cmat_d = nc.dram_tensor("cmat", [9, 5], F32, kind="ExternalInput")
    y_d = nc.dram_tensor("y", [NS, MD], F32, kind="ExternalOutput")

    with tile.TileContext(nc) as tc, ExitStack() as ctx:
        consts = ctx.enter_context(tc.tile_pool(name="consts", bufs=1))
        p34 = ctx.enter_context(tc.tile_pool(name="p34", bufs=1))
        p3p = ctx.enter_context(tc.tile_pool(name="p3p", bufs=2))
        dram_ag = ctx.enter_context(tc.tile_pool(name="dram_ag", bufs=1, space="DRAM"))
        dram_rs = ctx.enter_context(tc.tile_pool(name="dram_rs", bufs=1, space="DRAM"))
        # single PSUM pool; bank budget (8 banks):
        #   br   [128,1024] f32 = 2 banks x 2 bufs = 4   (branch; phase-4 rA)
        #   tr   [128,<=512]    = 1 bank  x 2 bufs = 2   (transposes; phase-4 rB)
        #   misc [128,<=512]    = 1 bank  x 2 bufs = 2   (setup, gate, logits)
        psum = ctx.enter_context(tc.tile_pool(name="psum", bufs=2, space="PSUM"))

        p1ctx = ExitStack()
        p1 = p1ctx.enter_context(tc.tile_pool(name="p1", bufs=1))
        st1 = p1ctx.enter_context(tc.tile_pool(name="st1", bufs=2))
        blendp = p1ctx.enter_context(tc.tile_pool(name="blendp", bufs=2))

        # ---------------- Setup A: tiny constants ----------------
        idf = consts.tile([128, 128], F32)
        make_identity(nc, idf)
        idb = consts.tile([128, 128], BF16)
        nc.any.tensor_copy(idb, idf)
        ones_lhs = consts.tile([1, 128], BF16)
        nc.vector.memset(ones_lhs, 1.0)
        ones_f = consts.tile([1, 128], F32)
        nc.vector.memset(ones_f, 1.0)
        idf1 = consts.tile([1, 1], F32)
        nc.vector.memset(idf1, 1.0)

        # softmax(act_w); polynomial coefs = wts @ cmat, broadcast to [128, 5]
        aw = consts.tile([1, 9], F32)
        nc.sync.dma_start(out=aw, in_=actw_d.ap()[None])
        aw_negmax = consts.tile([1, 1], F32)
        nc.vector.tensor_reduce(aw_negmax, aw, AX.X, ALU.max, negate=True)
        aw_exp = consts.tile([1, 9], F32)
        nc.scalar.activation(aw_exp, aw, AF.Exp, bias=aw_negmax)
        aw_sum = consts.tile([1, 1], F32)
        nc.vector.tensor_reduce(aw_sum, aw_exp, AX.X, ALU.add)
        aw_rec = consts.tile([1, 1], F32)
        nc.vector.reciprocal(aw_rec, aw_sum)
        wts_row = consts.tile([1, 9], F32)
        nc.vector.tensor_scalar_mul(wts_row, aw_exp, aw_rec)
        wtsT_ps = psum.tile([9, 1], F32, tag="tr")
        nc.tensor.transpose(wtsT_ps, wts_row, idf1)
        wtsT = consts.tile([9, 1], F32)
        nc.any.tensor_copy(wtsT, wtsT_ps)
        cmat_sb = consts.tile([9, 5], F32)
        nc.sync.dma_start(out=cmat_sb, in_=cmat_d[:, :])
        cw_ps = psum.tile([1, 512], F32, tag="tr")
        nc.tensor.matmul(cw_ps[:, 0:5], wtsT, cmat_sb, start=True, stop=True)
        cw_row = consts.tile([1, 5], F32)
        nc.any.tensor_copy(cw_row, cw_ps[:, 0:5])
        bc_ps = psum.tile([128, 512], F32, tag="tr")
        nc.tensor.matmul(bc_ps[:, 0:5], ones_f, cw_row, start=True, stop=True)
        coefs = consts.tile([128, 5], F32)
        nc.any.tensor_copy(coefs, bc_ps[:, 0:5])

        # bias rows -> bf16 via casting DMA (branch bias b-major ++ gate bias)
        bias_b = consts.tile([1, UBF + NB], BF16)
        nc.gpsimd.dma_start(out=bias_b, in_=bias_d.ap()[None])

        # ---------------- Setup A2: connectivity (replicated) --------------
        na_sb = consts.tile([128, UK], F32)
        nc.sync.dma_start(out=na_sb, in_=na_d.ap().rearrange("(t p) -> p t", p=128))
        cw1_sb = consts.tile([128, UK, 32], F32)
        nc.sync.dma_start(out=cw1_sb,
                          in_=cw1_d.ap().rearrange("(t p) c -> p t c", p=128))
        h_ps = psum.tile([1, 512], F32, tag="tr")
        for t in range(UK):
            nc.tensor.matmul(h_ps[:, 0:32], na_sb[:, t:t + 1], cw1_sb[:, t, :],
                             start=(t == 0), stop=(t == UK - 1))
        cb1_sb = consts.tile([1, 32], F32)
        nc.sync.dma_start(out=cb1_sb, in_=cb1_d.ap()[None])
        h_pre = consts.tile([1, 32], F32)
        nc.vector.tensor_add(h_pre, h_ps[:, 0:32], cb1_sb)
        h_sb = consts.tile([1, 32], F32)
        nc.scalar.activation(h_sb, h_pre, AF.Relu)
        hT_ps = psum.tile([32, 1], F32, tag="tr")
        nc.tensor.transpose(hT_ps, h_sb, idf1)
        hT_sb = consts.tile([32, 1], F32)
        nc.any.tensor_copy(hT_sb, hT_ps)
        cw2_sb = consts.tile([32, US], F32)
        nc.sync.dma_start(out=cw2_sb, in_=cw2_d[:, :])
        cn_ps = psum.tile([1, 512], F32, tag="tr")
        nc.tensor.matmul(cn_ps[:, 0:US], hT_sb, cw2_sb, start=True, stop=True)
        cb2_sb = consts.tile([1, US], F32)
        nc.sync.dma_start(out=cb2_sb, in_=cb2_d.ap()[None])
        cn_pre = consts.tile([1, US], F32)
        nc.vector.tensor_add(cn_pre, cn_ps[:, 0:US], cb2_sb)
        cn_sig = consts.tile([1, US], F32)
        nc.scalar.activation(cn_sig, cn_pre, AF.Sigmoid)
        mask_sb = consts.tile([1, US], F32)
        nc.sync.dma_start(out=mask_sb, in_=mask_d.ap()[None])
        cm_row = consts.tile([1, US], F32)
        nc.vector.tensor_mul(cm_row, cn_sig, mask_sb)
        cm_ps = psum.tile([128, 512], F32, tag="tr")
        nc.tensor.matmul(cm_ps[:, 0:US], ones_f, cm_row, start=True, stop=True)
        cm_bc = consts.tile([128, US], F32)
        nc.any.tensor_copy(cm_bc, cm_ps[:, 0:US])

        # ---------------- Setup B: w_mod (bf16, b-major) + gate_W ----------
        wmod_sb = p1.tile([128, DK, UBF + NB], BF16)
        for dk in range(DK):
            w_f = st1.tile([128, UBF + NB], F32, tag="ld1")
            nc.sync.dma_start(out=w_f, in_=wd_d[dk * 128:(dk + 1) * 128, :])
            d_f = st1.tile([128, UBF], F32, tag="ld1")
            nc.sync.dma_start(out=d_f, in_=dd_d[dk * 128:(dk + 1) * 128, :])
            sig_b = st1.tile([128, UBF], BF16, tag="tb1")
            nc.scalar.activation(sig_b, d_f, AF.Sigmoid)
            w_b = st1.tile([128, UBF], BF16, tag="tb1")
            nc.any.tensor_copy(w_b, w_f[:, 0:UBF])
            nc.any.tensor_mul(wmod_sb[:, dk, 0:UBF], w_b, sig_b)
            nc.any.tensor_copy(wmod_sb[:, dk, UBF:UBF + NB],
                               w_f[:, UBF:UBF + NB])

        # ------- Setup C: xT via PE transpose (bf16 via casting DMA) --------
        xT_sb = p1.tile([128, DK, N], BF16)
        for i in range(NT):
            x_b = st1.tile([128, D], BF16, tag="ldx")
            nc.gpsimd.dma_start(out=x_b, in_=x_d[i * 128:(i + 1) * 128, :])
            for dk in range(DK):
                tr_ps = psum.tile([128, 128], BF16, tag="tr")
                nc.tensor.transpose(tr_ps, x_b[:, dk * 128:(dk + 1) * 128], idb)
                nc.any.tensor_copy(xT_sb[:, dk, i * 128:(i + 1) * 128], tr_ps)

        # ---------------- Phase 3 prep loads (overlap with phase 1) --------
        rw_sb = p34.tile([128, UK, MS], BF16)
        for uk in range(UK):
            nc.gpsimd.dma_start(out=rw_sb[:, uk, :],
                                in_=rw_d[uk * 128:(uk + 1) * 128, :])
        mem_sb = p34.tile([128, MK, MD + 1], BF16)
        for mk in range(MK):
            nc.gpsimd.dma_start(out=mem_sb[:, mk, 0:MD],
                                in_=mem_d[mk * 128:(mk + 1) * 128, :])
            nc.vector.memset(mem_sb[:, mk, MD:MD + 1], 1.0)
        rb_sb = consts.tile([128, MK], F32)
        nc.sync.dma_start(out=rb_sb, in_=rb_d.ap().rearrange("(t p) -> p t", p=128))

        # -------- Phase 1b: branch matmul, z, poly blend; phase 3 lagged ----
        blendT_sb = p1.tile([128, 2, N], BF16)
        ag_outs = []
        expTs = []

        def emit_phase3(ch):
            bT_j = p3p.tile([128, UK, 512], BF16, tag="bT", name="bT_j")
            for uk in range(UK):
                nc.sync.dma_start(out=bT_j[:, uk, :],
                                  in_=ag_outs[ch][uk * 128:(uk + 1) * 128, :])
            expT_t = p3p.tile([128, MK, 512], BF16, tag="expT", bufs=4,
                              name="expT_t")
            for mk in range(MK):
                l_ps = psum.tile([128, 512], F32, tag="tr", name="l_ps")
                for uk in range(UK):
                    nc.tensor.matmul(l_ps,
                                     rw_sb[:, uk, mk * 128:(mk + 1) * 128],
                                     bT_j[:, uk, :],
                                     start=(uk == 0), stop=(uk == UK - 1))
                nc.scalar.activation(expT_t[:, mk, :], l_ps, AF.Exp,
                                     bias=rb_sb[:, mk:mk + 1])
            expTs.append(expT_t)

        for i in range(NT):
            nsl = slice(i * 128, (i + 1) * 128)
            br_ps = psum.tile([128, UBF + NB], F32, tag="br")
            for (c0, c1_) in [(0, 512), (512, 1024), (1024, 1028)]:
                for dk in range(DK):
                    nc.tensor.matmul(br_ps[:, c0:c1_],
                                     xT_sb[:, dk, nsl],
                                     wmod_sb[:, dk, c0:c1_],
                                     start=(dk == 0), stop=False)
                nc.tensor.matmul(br_ps[:, c0:c1_], ones_lhs,
                                 bias_b[:, c0:c1_],
                                 start=False, stop=True)
            # gate softmax on br_ps[:, 1024:1028]
            g_negmax = blendp.tile([128, 1], F32, tag="g1")
            nc.vector.tensor_reduce(g_negmax, br_ps[:, UBF:UBF + NB], AX.X,
                                    ALU.max, negate=True)
            g_exp = blendp.tile([128, NB], F32, tag="g2")
            nc.scalar.activation(g_exp, br_ps[:, UBF:UBF + NB], AF.Exp,
                                 bias=g_negmax)
            g_sum = blendp.tile([128, 1], F32, tag="g3")
            nc.vector.tensor_reduce(g_sum, g_exp, AX.X, ALU.add)
            g_rec = blendp.tile([128, 1], F32, tag="g4")
            nc.vector.reciprocal(g_rec, g_sum)
            gate_sb = blendp.tile([128, NB], F32, tag="g5")
            nc.vector.tensor_scalar_mul(gate_sb, g_exp, g_rec)
            # z = sum_b gate_b * branch_b   (contiguous b-major slices)
            zt0 = blendp.tile([128, US], F32, tag="t0")
            nc.any.tensor_scalar_mul(zt0, br_ps[:, 0:US], gate_sb[:, 0:1])
            zt1 = blendp.tile([128, US], F32, tag="t1")
            nc.any.tensor_scalar_mul(zt1, br_ps[:, US:2 * US], gate_sb[:, 1:2])
            zt2 = blendp.tile([128, US], F32, tag="t2")
            nc.any.tensor_scalar_mul(zt2, br_ps[:, 2 * US:3 * US],
                                     gate_sb[:, 2:3])
            zt3 = blendp.tile([128, US], F32, tag="t3", bufs=1)
            nc.any.tensor_scalar_mul(zt3, br_ps[:, 3 * US:4 * US],
                                     gate_sb[:, 3:4])
            z01 = blendp.tile([128, US], F32, tag="t0")
            nc.any.tensor_add(z01, zt0, zt1)
            z23 = blendp.tile([128, US], F32, tag="t2")
            nc.any.tensor_add(z23, zt2, zt3)
            z_sb = blendp.tile([128, US], F32, tag="t1")
            nc.any.tensor_add(z_sb, z01, z23)
            # a = relu(z * conn * mask)
            zc = blendp.tile([128, US], F32, tag="t0")
            nc.any.tensor_mul(zc, z_sb, cm_bc)
            a_sb = blendp.tile([128, US], F32, tag="ta")
            nc.any.tensor_scalar_max(a_sb, zc, 0.0)
            # blend via degree-4 Horner (per-partition scalar coefs)
            hp = blendp.tile([128, US], F32, tag="t2")
            nc.any.tensor_scalar(hp, a_sb, coefs[:, 4:5], coefs[:, 3:4],
                                 ALU.mult, ALU.add)
            hq = blendp.tile([128, US], F32, tag="t3", bufs=1)
            nc.any.tensor_mul(hq, hp, a_sb)
            hr = blendp.tile([128, US], F32, tag="t2")
            nc.any.tensor_scalar_add(hr, hq, coefs[:, 2:3])
            hs = blendp.tile([128, US], F32, tag="t3", bufs=1)
            nc.any.tensor_mul(hs, hr, a_sb)
            ht = blendp.tile([128, US], F32, tag="t2")
            nc.any.tensor_scalar_add(ht, hs, coefs[:, 1:2])
            hu = blendp.tile([128, US], F32, tag="t3", bufs=1)
            nc.any.tensor_mul(hu, ht, a_sb)
            blend_b16 = blendp.tile([128, US], BF16, tag="bb", bufs=1)
            nc.any.tensor_scalar_add(blend_b16, hu, coefs[:, 0:1])
            for uh in range(2):
                trb_ps = psum.tile([128, 128], BF16, tag="tr")
                nc.tensor.transpose(trb_ps,
                                    blend_b16[:, uh * 128:(uh + 1) * 128], idb)
                nc.any.tensor_copy(blendT_sb[:, uh, nsl], trb_ps)
            if i % 4 == 3:
                # issue AllGather for this 512-col chunk; run phase 3 for the
                # PREVIOUS chunk (its AllGather has had a 4-tile window to
                # land, so the in-order PE stream does not stall on it)
                ch = i // 4
                csl = slice(ch * 512, (ch + 1) * 512)
                agi = dram_ag.tile([US, 512], BF16, name=f"ag_in{ch}",
                                   tag=f"agi{ch}")
                for uh in range(2):
                    nc.sync.dma_start(out=agi[uh * 128:(uh + 1) * 128, :],
                                      in_=blendT_sb[:, uh, csl])
                ago = dram_ag.tile([U, 512], BF16, name=f"ag_out{ch}",
                                   tag=f"ago{ch}", addr_space="Shared")
                nc.gpsimd.collective_compute(
                    "AllGather", ALU.bypass,
                    replica_groups=[list(range(KC))],
                    ins=[agi.opt()], outs=[ago.opt()],
                )
                ag_outs.append(ago)
                if ch >= 1:
                    emit_phase3(ch - 1)

        emit_phase3(3)

        # phase-1 SBUF pools released; later pools can reuse their space
        p1ctx.close()
        p4p = ctx.enter_context(tc.tile_pool(name="p4p", bufs=2))

        # ---------------- Phase 4 + split ReduceScatter + epilogue ---------
        rs_outs = []

        def emit_epilogue(ch):
            e_f = p4p.tile([64, MD + 1], F32, tag="ef", bufs=1, name="e_f")
            nc.gpsimd.dma_start(out=e_f, in_=rs_outs[ch][:, :])
            s_rec = p4p.tile([64, 1], F32, tag="sr", name="s_rec")
            nc.vector.reciprocal(s_rec, e_f[:, MD:MD + 1])
            y_t = p4p.tile([64, MD], F32, tag="yt", bufs=1, name="y_t")
            nc.any.tensor_scalar_mul(y_t, e_f[:, 0:MD], s_rec)
            nc.gpsimd.dma_start(out=y_d[ch * 64:(ch + 1) * 64, :], in_=y_t)

        for ch in range(4):
            expT_t = expTs[ch]
            rs_inj = dram_rs.tile([512, MD + 1], F32, name=f"rs_in{ch}",
                                  tag=f"rsi{ch}")
            for sj in range(4):
                jsl = slice(sj * 128, (sj + 1) * 128)
                r_ps = psum.tile([128, UBF + NB], F32, tag="br")
                for (c0, c1_) in [(0, 512), (512, 1024), (1024, 1025)]:
                    for mk in range(MK):
                        nc.tensor.matmul(r_ps[:, c0:c1_], expT_t[:, mk, jsl],
                                         mem_sb[:, mk, c0:c1_],
                                         start=(mk == 0), stop=(mk == MK - 1))
                r_sb = p4p.tile([128, MD + 1], F32, tag="rsb", bufs=1)
                nc.any.tensor_copy(r_sb, r_ps[:, 0:MD + 1])
                nc.sync.dma_start(out=rs_inj[sj * 128:(sj + 1) * 128, :],
                                  in_=r_sb)

            # core k receives global output rows [ch*512+k*64, ch*512+(k+1)*64)
            rs_out_j = dram_rs.tile([N // 32, MD + 1], F32,
                                    name=f"rs_out{ch}", tag=f"rso{ch}")
            nc.gpsimd.collective_compute(
                "ReduceScatter", ALU.add,
                replica_groups=[list(range(KC))],
                ins=[rs_inj.opt()], outs=[rs_out_j.opt()],
            )
            rs_outs.append(rs_out_j)
            if ch >= 2:
                emit_epilogue(ch - 2)
        emit_epilogue(2)
        emit_epilogue(3)

    nc.compile()
    return nc


def _make_in_maps(inputs):
    x = np.ascontiguousarray(np.asarray(inputs["x"], np.float32))
    w = np.asarray(inputs["w"], np.float32)
    delay = np.asarray(inputs["delay"], np.float32)
    b = np.asarray(inputs["b"], np.float32)
    gate_W = np.ascontiguousarray(np.asarray(inputs["gate_W"], np.float32))
    gate_b = np.asarray(inputs["gate_b"], np.float32)
    na = np.ascontiguousarray(np.asarray(inputs["neuron_avg"], np.float32))
    cw1 = np.ascontiguousarray(np.asarray(inputs["conn_W1"], np.float32))
    cb1 = np.ascontiguousarray(np.asarray(inputs["conn_b1"], np.float32))
    cw2 = np.asarray(inputs["conn_W2"], np.float32)
    cb2 = np.asarray(inputs["conn_b2"], np.float32)
    mask = np.asarray(inputs["mask"], np.float32)
    actw = np.ascontiguousarray(np.asarray(inputs["act_w"], np.float32))
    read_W = np.asarray(inputs["read_W"], np.float32)
    read_b = np.asarray(inputs["read_b"], np.float32)
    mem = np.asarray(inputs["memory"], np.float32)
    cmat = _CMAT

    in_maps = []
    for k in range(KC):
        us, ue = k * US, (k + 1) * US
        ms, me = k * MS, (k + 1) * MS
        bias_row = np.concatenate([b[us:ue].T.reshape(-1),
                                   gate_b]).astype(np.float32)
        in_maps.append({
            "x": x,
            "wd": np.ascontiguousarray(np.concatenate(
                [w[:, us:ue, :].transpose(0, 2, 1).reshape(D, UBF), gate_W],
                axis=1)),
            "dd": np.ascontiguousarray(
                delay[:, us:ue, :].transpose(0, 2, 1).reshape(D, UBF)),
            "bias": np.ascontiguousarray(bias_row),
            "na": na,
            "cw1": cw1,
            "cb1": cb1,
            "cw2": np.ascontiguousarray(cw2[:, us:ue]),
            "cb2": np.ascontiguousarray(cb2[us:ue]),
            "maskv": np.ascontiguousarray(mask[us:ue]),
            "actw": actw,
            "rw": np.ascontiguousarray(read_W[:, ms:me]),
            "rb": np.ascontiguousarray(read_b[ms:me]),
            "mem": np.ascontiguousarray(mem[ms:me, :]),
            "cmat": cmat,
        })
    return in_maps


def kernel(**inputs) -> np.ndarray:
    if "nc" not in _cache:
        _cache["nc"] = _build()
    nc = _cache["nc"]
    in_maps = _make_in_maps(inputs)
    res = run_bass_kernel_spmd(nc, in_maps, core_ids=list(range(KC)))
    out = np.empty((N, MD), np.float32)
    for k in range(KC):
        yk = res.results[k]["y"]
        for j in range(4):
            out[j * 512 + k * 64:j * 512 + (k + 1) * 64] = \
                yk[j * 64:(j + 1) * 64]
    return out



# revision 5
# speedup vs baseline: 1.6879x; 1.6879x over previous
"""Trainium2 Bass kernel for nn_PlasticityModelMoE (8-core SPMD).

Strategy (v2, fp8):
  Host precomputes all weight transforms: wmod = w*sigmoid(delay)*conn*mask*64
  (b-major, fp8e4m3) concatenated with gate_W*64; x pre-transposed to fp8 xT;
  read_W shard scaled *64 fp8; memory shard (+ones col) fp16; connectivity MLP
  and activation-blend polynomial coefs (scaled *16) computed on host.
  Device: phase 1 (units tensor-parallel, 256/core): branch+gate logits via
  fp8 DoubleRow matmuls (K=256/step), gate softmax, z-combine, relu, degree-4
  Horner blend -> fp8 blendT (*16).  Per 512-col batch chunk: fp8 AllGather of
  blendT, then phase 3 (memory-rows tensor-parallel, 1024/core): logitsT =
  rw8 x bT via fp8 DoubleRow, exp (descale 2^-10, +read_b) -> fp16 expT.
  Phase 4: [read_partial | s] = E @ [mem | 1] in fp16, fp16 ReduceScatter over
  batch rows, divide by s -> each core emits its 256-row f32 output shard.
  A dummy 64B AllGather at t=0 absorbs the first-collective rendezvous.
"""
import numpy as np
from contextlib import ExitStack

import concourse.bass as bass
import concourse.mybir as mybir
import concourse.tile as tile
from concourse import bacc
from concourse.bass_utils import run_bass_kernel_spmd
from concourse.masks import make_identity

F32 = mybir.dt.float32
BF16 = mybir.dt.bfloat16
F16 = mybir.dt.float16
F8 = mybir.dt.float8e4
AF = mybir.ActivationFunctionType
ALU = mybir.AluOpType
AX = mybir.AxisListType
PM = mybir.MatmulPerfMode

KC = 8
N, D, U, NB, M, MD = 2048, 1024, 2048, 4, 8192, 1024
US = U // KC          # 256 units per core
MS = M // KC          # 1024 memory rows per core
NS = N // KC          # 256 output rows per core
NT = N // 128         # 16 batch tiles
DK = D // 128         # 8 k-tiles over D
DR = DK // 2          # 4 DoubleRow steps over D
UK = U // 128         # 16 k-tiles over U
UR = UK // 2          # 8 DoubleRow steps over U
MK = MS // 128        # 8 k-tiles over memory shard
UBF = US * NB         # 1024 branch columns per core
CH, CW = 4, 512       # batch chunks for collectives
SC_W = 64.0           # fp8 weight scale (2^6)
SC_B = 16.0           # fp8 blend scale (2^4)
DESC = 1.0 / (SC_W * SC_B)

_CMAT = np.array([
    [5.0000238e-01, 2.4987496e-01, 1.0582031e-03, -2.4046743e-02, 4.1678566e-03],
    [0.0, 1.0, 0.0, 0.0, 0.0],
    [-7.2632770e-06, 9.9976927e-01, 9.2018498e-03, -3.9401752e-01, 1.4669961e-01],
    [0.0, 1.0, 0.0, 0.0, 0.0],
    [8.6798245e-06, 4.9957812e-01, 2.5321743e-01, -8.1970906e-03, -1.3558048e-02],
    [3.9388153e-05, 4.9807969e-01, 4.1364601e-01, -3.7666172e-02, -3.2796454e-02],
    [0.0, 1.0507009873554805, 0.0, 0.0, 0.0],
    [3.1482985e-05, 5.9846270e-01, 3.3178753e-01, -4.6201140e-02, -1.9015398e-02],
    [0.0, 0.0, 0.0, 0.0, 0.0],
], dtype=np.float64)

_cache = {}


def _build():
    nc = bacc.Bacc(num_devices=KC)

    xt_d = nc.dram_tensor("xt", [D, N], F8, kind="ExternalInput")
    wd_d = nc.dram_tensor("wd", [D, UBF + NB], F8, kind="ExternalInput")
    bias_d = nc.dram_tensor("bias", [UBF + NB], BF16, kind="ExternalInput")
    coefs_d = nc.dram_tensor("coefs", [128, 5], F32, kind="ExternalInput")
    rw_d = nc.dram_tensor("rw", [U, MS], F8, kind="ExternalInput")
    rb_d = nc.dram_tensor("rb", [MS], F32, kind="ExternalInput")
    mem_d = nc.dram_tensor("mem", [MS, MD + 1], F16, kind="ExternalInput")
    y_d = nc.dram_tensor("y", [NS, MD], F32, kind="ExternalOutput")

    with tile.TileContext(nc) as tc, ExitStack() as ctx:
        consts = ctx.enter_context(tc.tile_pool(name="consts", bufs=1))
        big = ctx.enter_context(tc.tile_pool(name="big", bufs=1))
        p3p = ctx.enter_context(tc.tile_pool(name="p3p", bufs=2))
        p4p = ctx.enter_context(tc.tile_pool(name="p4p", bufs=2))
        blendp = ctx.enter_context(tc.tile_pool(name="blendp", bufs=2))
        dram_ag = ctx.enter_context(tc.tile_pool(name="dram_ag", bufs=1, space="DRAM"))
        dram_rs = ctx.enter_context(tc.tile_pool(name="dram_rs", bufs=1, space="DRAM"))
        # PSUM budget (8 banks): br [128,1028] f32 ~2 banks x 2 bufs,
        # tr [128,<=512] 1 bank x 2 bufs
        psum = ctx.enter_context(tc.tile_pool(name="psum", bufs=2, space="PSUM"))

        # ---------- tiny consts ----------
        idf = consts.tile([128, 128], F32)
        make_identity(nc, idf)
        idb = consts.tile([128, 128], BF16)
        nc.any.tensor_copy(idb, idf)
        ones_lhs = consts.tile([1, 128], BF16)
        nc.vector.memset(ones_lhs, 1.0)
        bias_b = consts.tile([1, UBF + NB], BF16)
        nc.sync.dma_start(out=bias_b, in_=bias_d.ap()[None])
        coefs = consts.tile([128, 5], F32)
        nc.sync.dma_start(out=coefs, in_=coefs_d[:, :])
        rb_sb = consts.tile([128, MK], F32)
        nc.sync.dma_start(out=rb_sb, in_=rb_d.ap().rearrange("(t p) -> p t", p=128))

        # ---------- dummy collective to absorb first-cc rendezvous ----------
        dummy_sb = consts.tile([1, 16], F32)
        nc.vector.memset(dummy_sb, 0.0)
        dummy_in = dram_ag.tile([1, 16], F32, name="dummy_in", tag="dmi")
        nc.gpsimd.dma_start(out=dummy_in, in_=dummy_sb)
        dummy_out = dram_ag.tile([KC, 16], F32, name="dummy_out", tag="dmo",
                                 addr_space="Shared")
        nc.gpsimd.collective_compute(
            "AllGather", ALU.bypass, replica_groups=[list(range(KC))],
            ins=[dummy_in.opt()], outs=[dummy_out.opt()])

        # ---------- big input loads (sync queue, priority order) ----------
        wm = big.tile([128, DK, UBF + NB], F8)
        nc.sync.dma_start(out=wm, in_=wd_d.ap().rearrange("(t p) c -> p t c", p=128))
        xT = big.tile([128, DK, N], F8)
        for c in range(CH):
            csl = slice(c * CW, (c + 1) * CW)
            nc.sync.dma_start(
                out=xT[:, :, csl],
                in_=xt_d.ap()[:, csl].rearrange("(t p) n -> p t n", p=128))
        rw8 = big.tile([128, UK, MS], F8)
        nc.sync.dma_start(out=rw8, in_=rw_d.ap().rearrange("(t p) m -> p t m", p=128))
        mem16 = big.tile([128, MK, MD + 1], F16)
        nc.sync.dma_start(out=mem16, in_=mem_d.ap().rearrange("(t p) c -> p t c", p=128))

        blendT = big.tile([128, 2, N], F8)
        ag_outs = []
        expTs = []
        rs_outs = []

        def emit_tile(i):
            nsl = slice(i * 128, (i + 1) * 128)
            br = psum.tile([128, UBF + NB], F32, tag="br", name="br")
            for (c0, c1) in [(0, 512), (512, 1024), (1024, 1028)]:
                for s in range(DR):
                    nc.tensor.matmul(br[:, c0:c1],
                                     xT[:, 2 * s:2 * s + 2, nsl],
                                     wm[:, 2 * s:2 * s + 2, c0:c1],
                                     start=(s == 0), stop=False,
                                     perf_mode=PM.DoubleRow)
                nc.tensor.matmul(br[:, c0:c1], ones_lhs, bias_b[:, c0:c1],
                                 start=False, stop=True, skip_group_check=True)
            # gate softmax on br[:, 1024:1028] (logits are *SC_W; exp safe
            # without max-sub: true |logit| <~ 4)
            g_exp = blendp.tile([128, NB], F32, tag="g1")
            nc.scalar.activation(g_exp, br[:, UBF:UBF + NB], AF.Exp,
                                 scale=1.0 / SC_W)
            g_sum = blendp.tile([128, 1], F32, tag="g2")
            nc.vector.tensor_reduce(g_sum, g_exp, AX.X, ALU.add)
            g_rec = blendp.tile([128, 1], F32, tag="g3")
            nc.vector.reciprocal(g_rec, g_sum)
            gate = blendp.tile([128, NB], F32, tag="g4")
            nc.any.tensor_scalar(gate, g_exp, g_rec[:, 0:1], 1.0 / SC_W,
                                 ALU.mult, ALU.mult)
            # z = sum_b gate_b * branch_b  (bf16 pipeline)
            zt0 = blendp.tile([128, US], BF16, tag="t0")
            nc.any.tensor_scalar_mul(zt0, br[:, 0:US], gate[:, 0:1])
            zt1 = blendp.tile([128, US], BF16, tag="t1")
            nc.any.tensor_scalar_mul(zt1, br[:, US:2 * US], gate[:, 1:2])
            zt2 = blendp.tile([128, US], BF16, tag="t2")
            nc.any.tensor_scalar_mul(zt2, br[:, 2 * US:3 * US], gate[:, 2:3])
            zt3 = blendp.tile([128, US], BF16, tag="t3")
            nc.any.tensor_scalar_mul(zt3, br[:, 3 * US:4 * US], gate[:, 3:4])
            z01 = blendp.tile([128, US], BF16, tag="t0")
            nc.any.tensor_add(z01, zt0, zt1)
            z23 = blendp.tile([128, US], BF16, tag="t2")
            nc.any.tensor_add(z23, zt2, zt3)
            z_sb = blendp.tile([128, US], BF16, tag="t1")
            nc.any.tensor_add(z_sb, z01, z23)
            a_sb = blendp.tile([128, US], BF16, tag="ta")
            nc.any.tensor_scalar_max(a_sb, z_sb, 0.0)
            # blend*16 via degree-4 Horner (coefs prescaled *16)
            hp = blendp.tile([128, US], BF16, tag="t0")
            nc.any.tensor_scalar(hp, a_sb, coefs[:, 4:5], coefs[:, 3:4],
                                 ALU.mult, ALU.add)
            hq = blendp.tile([128, US], BF16, tag="t2")
            nc.any.tensor_mul(hq, hp, a_sb)
            hr = blendp.tile([128, US], BF16, tag="t0")
            nc.any.tensor_scalar_add(hr, hq, coefs[:, 2:3])
            hs = blendp.tile([128, US], BF16, tag="t2")
            nc.any.tensor_mul(hs, hr, a_sb)
            ht = blendp.tile([128, US], BF16, tag="t0")
            nc.any.tensor_scalar_add(ht, hs, coefs[:, 1:2])
            hu = blendp.tile([128, US], BF16, tag="t2")
            nc.any.tensor_mul(hu, ht, a_sb)
            blend16 = blendp.tile([128, US], BF16, tag="bb")
            nc.any.tensor_scalar_add(blend16, hu, coefs[:, 0:1])
            for uh in range(2):
                trb = psum.tile([128, 128], BF16, tag="tr", name="trb")
                nc.tensor.transpose(trb, blend16[:, uh * 128:(uh + 1) * 128], idb)
                nc.any.tensor_copy(blendT[:, uh, nsl], trb)

        def emit_ag(ch):
            csl = slice(ch * CW, (ch + 1) * CW)
            agi = dram_ag.tile([US, CW], F8, name=f"ag_in{ch}", tag=f"agi{ch}")
            for uh in range(2):
                nc.gpsimd.dma_start(out=agi[uh * 128:(uh + 1) * 128, :],
                                    in_=blendT[:, uh, csl])
            ago = dram_ag.tile([U, CW], F8, name=f"ag_out{ch}", tag=f"ago{ch}",
                               addr_space="Shared")
            nc.gpsimd.collective_compute(
                "AllGather", ALU.bypass, replica_groups=[list(range(KC))],
                ins=[agi.opt()], outs=[ago.opt()])
            ag_outs.append(ago)

        def emit_phase3(ch):
            bT = p3p.tile([128, UK, CW], F8, tag="bT", name="bT")
            nc.sync.dma_start(
                out=bT, in_=ag_outs[ch][:, :].rearrange("(t p) n -> p t n", p=128))
            expT = p3p.tile([128, MK, CW], F16, tag="expT", name="expT")
            for mk in range(MK):
                l_ps = psum.tile([128, CW], F32, tag="tr", name="l_ps")
                for s in range(UR):
                    nc.tensor.matmul(l_ps,
                                     rw8[:, 2 * s:2 * s + 2,
                                         mk * 128:(mk + 1) * 128],
                                     bT[:, 2 * s:2 * s + 2, :],
                                     start=(s == 0), stop=(s == UR - 1),
                                     perf_mode=PM.DoubleRow)
                nc.scalar.activation(expT[:, mk, :], l_ps, AF.Exp,
                                     bias=rb_sb[:, mk:mk + 1], scale=DESC)
            expTs.append(expT)

        def emit_phase4(ch):
            expT = expTs[ch]
            rs_inj = dram_rs.tile([CW, MD + 1], F16, name=f"rs_in{ch}",
                                  tag=f"rsi{ch}")
            for sj in range(4):
                jsl = slice(sj * 128, (sj + 1) * 128)
                r_ps = psum.tile([128, UBF + NB], F32, tag="br", name="r_ps")
                for (c0, c1) in [(0, 512), (512, 1024), (1024, 1025)]:
                    for mk in range(MK):
                        nc.tensor.matmul(r_ps[:, c0:c1], expT[:, mk, jsl],
                                         mem16[:, mk, c0:c1],
                                         start=(mk == 0), stop=(mk == MK - 1))
                r_sb = p4p.tile([128, MD + 1], F16, tag="rsb")
                nc.any.tensor_copy(r_sb, r_ps[:, 0:MD + 1])
                nc.gpsimd.dma_start(out=rs_inj[sj * 128:(sj + 1) * 128, :],
                                    in_=r_sb)
            rs_out = dram_rs.tile([CW // KC, MD + 1], F16, name=f"rs_out{ch}",
                                  tag=f"rso{ch}")
            nc.gpsimd.collective_compute(
                "ReduceScatter", ALU.add, replica_groups=[list(range(KC))],
                ins=[rs_inj.opt()], outs=[rs_out.opt()])
            rs_outs.append(rs_out)

        def emit_epilogue(ch):
            e_f = p4p.tile([CW // KC, MD + 1], F16, tag="ef", name="e_f")
            nc.scalar.dma_start(out=e_f, in_=rs_outs[ch][:, :])
            s_rec = p4p.tile([CW // KC, 1], F32, tag="sr", name="s_rec")
            nc.vector.reciprocal(s_rec, e_f[:, MD:MD + 1])
            y_t = p4p.tile([CW // KC, MD], F32, tag="yt", name="y_t")
            nc.any.tensor_scalar_mul(y_t, e_f[:, 0:MD], s_rec[:, 0:1])
            nc.gpsimd.dma_start(out=y_d[ch * 64:(ch + 1) * 64, :], in_=y_t)

        # ---------- pipelined emission ----------
        for ch in range(CH):
            for it in range(4):
                emit_tile(ch * 4 + it)
            emit_ag(ch)
            if ch >= 1:
                emit_phase3(ch - 1)
                emit_phase4(ch - 1)
            if ch >= 2:
                emit_epilogue(ch - 2)
        emit_phase3(3)
        emit_phase4(3)
        emit_epilogue(2)
        emit_epilogue(3)

    nc.compile()
    return nc


def _sigmoid(v):
    return 1.0 / (1.0 + np.exp(-v))


def _make_in_maps(inputs):
    F8NP = mybir.dt.np(F8)
    x = np.asarray(inputs["x"], np.float32)
    w = np.asarray(inputs["w"], np.float64)
    delay = np.asarray(inputs["delay"], np.float64)
    b = np.asarray(inputs["b"], np.float64)
    gate_W = np.asarray(inputs["gate_W"], np.float64)
    gate_b = np.asarray(inputs["gate_b"], np.float64)
    na = np.asarray(inputs["neuron_avg"], np.float64)
    cw1 = np.asarray(inputs["conn_W1"], np.float64)
    cb1 = np.asarray(inputs["conn_b1"], np.float64)
    cw2 = np.asarray(inputs["conn_W2"], np.float64)
    cb2 = np.asarray(inputs["conn_b2"], np.float64)
    mask = np.asarray(inputs["mask"], np.float64)
    actw = np.asarray(inputs["act_w"], np.float64)
    read_W = np.asarray(inputs["read_W"], np.float32)
    read_b = np.asarray(inputs["read_b"], np.float32)
    mem = np.asarray(inputs["memory"], np.float32)

    # connectivity MLP (batch-independent, one row)
    h = np.maximum(na[None, :] @ cw1 + cb1, 0.0)
    conn = (_sigmoid(h @ cw2 + cb2)[0] * mask)            # [U]
    # activation-blend polynomial coefs, prescaled for fp8 blend
    e = np.exp(actw - actw.max())
    wts = e / e.sum()
    coefs = (wts @ _CMAT) * SC_B                          # [5]
    coefs_bc = np.ascontiguousarray(
        np.broadcast_to(coefs.astype(np.float32), (128, 5)))

    xt8 = np.ascontiguousarray(x.T).astype(F8NP)          # [D, N]
    wmod = w * _sigmoid(delay)                            # [D, U, NB]
    wmod = wmod * conn[None, :, None] * SC_W

    in_maps = []
    for k in range(KC):
        us, ue = k * US, (k + 1) * US
        ms, me = k * MS, (k + 1) * MS
        wd8 = np.concatenate(
            [wmod[:, us:ue, :].transpose(0, 2, 1).reshape(D, UBF),
             gate_W * SC_W], axis=1).astype(F8NP)
        bias_row = np.concatenate(
            [(b[us:ue] * conn[us:ue, None]).T.reshape(-1), gate_b]) * SC_W
        mem_aug = np.concatenate(
            [mem[ms:me], np.ones((MS, 1), np.float32)], axis=1)
        in_maps.append({
            "xt": xt8,
            "wd": np.ascontiguousarray(wd8),
            "bias": bias_row.astype(mybir.dt.np(BF16)),
            "coefs": coefs_bc,
            "rw": np.ascontiguousarray(
                (read_W[:, ms:me] * np.float32(SC_W)).astype(F8NP)),
            "rb": np.ascontiguousarray(read_b[ms:me]),
            "mem": np.ascontiguousarray(mem_aug.astype(np.float16)),
        })
    return in_maps


def kernel(**inputs) -> np.ndarray:
    if "nc" not in _cache:
        _cache["nc"] = _build()
    nc = _cache["nc"]
    in_maps = _make_in_maps(inputs)
    res = run_bass_kernel_spmd(nc, in_maps, core_ids=list(range(KC)))
    out = np.empty((N, MD), np.float32)
    for k in range(KC):
        yk = res.results[k]["y"]
        for j in range(4):
            out[j * 512 + k * 64:j * 512 + (k + 1) * 64] = \
                yk[j * 64:(j + 1) * 64]
    return out
